# revision 41
# baseline (speedup 1.0000x reference)
"""Trainium2 Bass kernel for nn_ExpandEvecs.

Computes, for evecs [B=4, C=1, N=1024, K=16]:
    cube[b,l] = V[:, :l+1] @ V[:, :l+1]^T  (Gram expansion per level).

Every level is SYMMETRIC: the device computes only the upper
block-triangle (18 of 64 blocks of 128x128 per core-half) and the host
mirrors the strictly-lower blocks during unsharding. Output stored
bf16, upcast on host.

Architecture (v2): row-tiled PE + copy-balanced evacuation.
Contraction kk = l+1 <= 16 (plain bf16 inputs; the A/B split trick of
v1 is unnecessary: total err 0.19 vs 0.99 allowed), so every matmul
fits one 32-row group of the PE array; tile_position row tiling runs
the four groups' matmuls concurrently, making PE a non-factor (~10us
vs v1's 31us serial column stream). The binding resource is PSUM
evacuation: PSUM has exactly two read ports (Vector @0.96 GHz, Scalar
@1.2 GHz, 1 elem/lane/cycle, no 2x modes reach PSUM on TRN2), so the
2304 f32 cols/level cost ~1.45us/level minimum (scalar: AB 1024-col
copy ~1.11us + amortized E pair ~0.34; vector: C+D ~1.37us). The
orphan 256-col E strip batches ACROSS LEVEL PAIRS into one psum bank
(two matmuls fill halves; one copy + one store per pair into a
separate dram tensor) so its copy cost amortizes. PSUM = 4+2+1+1 = 8
banks exactly (a full level double-buffered needs 9 -- impossible).
Ordering rules (all measured, each worth 1-5us): AB matmuls LEAD the
PE queue (b2 slack means they never wait at the strict-FIFO head;
putting the b1 D chain first costs +5us), C goes last; on the copy
engines the tight b1 copies (E pair on scalar, D on vector) go before
the b2 copies. Stores ride both HWDGE rings (~320 GB/s each),
alternating per level; the last two levels store per-copy to shorten
the drain. Levels are independent fresh matmuls, emitted DESCENDING.

Measured 42.9-43.2us (fast machine state; a bimodal ~+7% slow state
exists) vs the 48.7-48.9us v1 baseline. Anatomy: ~3.3us const-init +
input cold start to first matmul (priority input slices ride the
gpsimd SWDGE queue -- the HWDGE rings dispatch ~1us later behind
framework DRAIN/SET_ORDERING/ACT_TABLE preamble), ~24us copy-bound
level stream at ~1.54us/level (floor 1.45: scalar AB+E2 1.46us,
vector C+D 1.37us), ~4us ring-pipeline store drain, ~8.2us fixed
framework teardown (full semaphore-file sweep, range(150,256),
invariant). E-pair stores must stay on the sync ring: a scalar-ring
dispatch there serializes with the pacing AB copy and costs ~2us
every other pair (measured, fixed).

Tested and rejected: gpsimd SWDGE for bulk stores (slower), fp8
storage (DMA is not the binding resource; PSUM-read cycles are),
E2-copy-after-AB (PE FIFO stall), equal-width 4-chain layouts (a
fully double-buffered level needs 9 banks), wider single-buffered
chains (mm->copy->mm round trip exceeds the 1.45us cadence), C/D
copies ordered after the b2 chains on vector (starves the b1 D
chain), per-block fp8 or diag-block triangle skipping (SPMD shape
mismatch / per-instr overhead exceeds the saving).
"""

import numpy as np
import ml_dtypes

import concourse.mybir as mybir
from concourse import bacc, bass
from concourse.tile import TileContext
from concourse.bass_utils import run_bass_kernel_spmd

B, C, N, K = 4, 1, 1024, 16
NCORES = 8
PACK = 2048            # st columns per level (A|B|C|D)

F32 = mybir.dt.float32
BF16 = mybir.dt.bfloat16
BF16_NP = ml_dtypes.bfloat16

# per core-half: 6 pieces as (block_row, col0, col1, stage_off, row_grp)
# chains: A(0:512)@0, B(512:1024)@32, C(1024:1536)@64,
#         D0(1536:1920)+D1(1920:2048)@96, E(pair-batched)@96
PIECES = [
    [(0, 0, 512, 0, 0), (0, 512, 1024, 512, 32), (4, 512, 1024, 1024, 64),
     (1, 640, 1024, 1536, 96), (3, 896, 1024, 1920, 96),
     (2, 768, 1024, -1, 96)],
    [(1, 128, 640, 0, 0), (2, 256, 768, 512, 32), (3, 384, 896, 1024, 64),
     (5, 640, 1024, 1536, 96), (7, 896, 1024, 1920, 96),
     (6, 768, 1024, -1, 96)],
]
# lhsT column slot per piece (each 128 wide in lt)
LSLOT = [0, 0, 0, 0, 1, 2]
# rhs column range per piece inside rt (per row-group packing)
RCOL = [(0, 512), (0, 512), (0, 512), (0, 384), (384, 512), (512, 768)]

_nc_cache = None


def _build():
    nc = bacc.Bacc(None, target_bir_lowering=False)
    lt_d = nc.declare_dram_parameter("lt", [128, 384], BF16, isOutput=False)
    rt_d = nc.declare_dram_parameter("rt", [128, 768], BF16, isOutput=False)
    out_d = nc.declare_dram_parameter("out", [K, 128, PACK], BF16,
                                      isOutput=True)
    # E columns, one row per level PAIR: [E(level hi) | E(level lo)]
    oute_d = nc.declare_dram_parameter("oute", [K // 2, 128, 512], BF16,
                                       isOutput=True)

    with TileContext(nc) as tc:
        with (
            tc.tile_pool(name="vpool", bufs=1) as vpool,
            tc.tile_pool(name="stage", bufs=6) as stage,
            tc.tile_pool(name="stagee", bufs=4) as stagee,
            tc.tile_pool(name="psum", bufs=1, space=bass.MemorySpace.PSUM) as psum,
        ):
            lt = vpool.tile([128, 384], BF16)
            rt = vpool.tile([128, 768], BF16)
            # split loads: A+B partitions land first so level K-1's
            # first matmuls start earlier. The priority loads ride the
            # gpsimd SWDGE queue; the HWDGE rings carry the rest and
            # arrive in time for the later chains.
            nc.gpsimd.dma_start(out=lt[0:48, :], in_=lt_d[0:48, :])
            nc.gpsimd.dma_start(out=rt[0:48, :], in_=rt_d[0:48, :])
            nc.sync.dma_start(out=rt[64:112, :], in_=rt_d[64:112, :])
            nc.scalar.dma_start(out=lt[64:112, :], in_=lt_d[64:112, :])

            psE = None
            stE = None
            for i, l in enumerate(range(K - 1, -1, -1)):
                kk = l + 1
                st = stage.tile([128, PACK], BF16, tag="st", name=f"st{l}")
                psAB = psum.tile([128, 1024], F32, tag="psAB", bufs=2,
                                 name=f"psAB{l}")
                psC = psum.tile([128, 512], F32, tag="psC", bufs=2,
                                name=f"psC{l}")
                psD = psum.tile([128, 512], F32, tag="psD", bufs=1,
                                name=f"psD{l}")
                if i % 2 == 0:
                    psE = psum.tile([128, 512], F32, tag="psE", bufs=1,
                                    name=f"psE{l}")
                    stE = stagee.tile([128, 512], BF16, tag="stE",
                                      name=f"stE{l}")

                def mm(ps, slot, r0, r1, g):
                    nc.tensor.matmul(
                        ps,
                        lhsT=lt[g:g + kk, 128 * slot:128 * (slot + 1)],
                        rhs=rt[g:g + kk, r0:r1],
                        start=True, stop=True,
                        tile_position=(g, 0),
                    )

                # PE issue order: AB leads (it feeds the scalar pacer
                # and its b2 slack means it never waits at the FIFO
                # head), then the tight b1 chains D/E, C last. Putting
                # D first measures +5us: its b1 wait at the FIFO head
                # blocks the AB matmuls behind it every level.
                mm(psAB[:, 0:512], 0, 0, 512, 0)
                mm(psAB[:, 512:1024], 0, 0, 512, 32)
                mm(psD[:, 0:384], 0, 0, 384, 96)
                mm(psD[:, 384:512], 1, 384, 512, 96)
                eoff = 256 * (i % 2)
                mm(psE[:, eoff:eoff + 256], 2, 512, 768, 96)
                mm(psC[:, :], 0, 0, 512, 64)

                # PSUM->SBUF bf16 casts: scalar AB (+ E pair), vector
                # D then C. Tight bufs=1 chains (E pair, D) are copied
                # FIRST on their engines; AB's b2 absorbs the wait.
                if i % 2 == 1:
                    nc.scalar.copy(stE[:], psE[:])
                nc.scalar.copy(st[:, 0:1024], psAB[:])
                nc.vector.tensor_copy(st[:, 1536:2048], psD[:])
                nc.vector.tensor_copy(st[:, 1024:1536], psC[:])

                # stores alternate rings per level for byte balance;
                # first and last levels store per-copy (fast ramp-in,
                # short drain)
                r_ab = nc.sync if i % 2 == 0 else nc.scalar
                r_cd = nc.scalar if i % 2 == 0 else nc.sync
                if i == K - 1:
                    # final level: 512-col chunks alternating rings so
                    # neither ring pipelines a long last transfer
                    for s in range(4):
                        ring = nc.sync if s % 2 == 0 else nc.scalar
                        ring.dma_start(out=out_d[l, :, 512 * s:512 * (s + 1)],
                                       in_=st[:, 512 * s:512 * (s + 1)])
                elif i == K - 2:
                    r_ab.dma_start(out=out_d[l, :, 0:1024],
                                   in_=st[:, 0:1024])
                    r_cd.dma_start(out=out_d[l, :, 1024:1536],
                                   in_=st[:, 1024:1536])
                    r_ab.dma_start(out=out_d[l, :, 1536:2048],
                                   in_=st[:, 1536:2048])
                else:
                    r_ab.dma_start(out=out_d[l, :, 0:1024],
                                   in_=st[:, 0:1024])
                    r_cd.dma_start(out=out_d[l, :, 1024:2048],
                                   in_=st[:, 1024:2048])

                if i % 2 == 1:
                    # close the E pair: one store (copy emitted above).
                    # Always the sync ring: a scalar-ring dispatch here
                    # serializes with the pacing AB copy (measured ~2us
                    # hiccup). Final pair splits across rings (past the
                    # last AB copy, no hazard).
                    if i == K - 1:
                        nc.sync.dma_start(out=oute_d[i // 2, :, 0:256],
                                          in_=stE[:, 0:256])
                        nc.scalar.dma_start(out=oute_d[i // 2, :, 256:512],
                                            in_=stE[:, 256:512])
                    else:
                        nc.sync.dma_start(out=oute_d[i // 2], in_=stE[:])

    nc.compile()
    return nc


def _get_nc():
    global _nc_cache
    if _nc_cache is None:
        _nc_cache = _build()
    return _nc_cache


def _prepare_in_maps(evecs: np.ndarray) -> list[dict]:
    in_maps = []
    for c in range(NCORES):
        b, h = divmod(c, 2)
        vt = np.ascontiguousarray(evecs[b, 0].T, dtype=np.float32)  # [K, N]
        a = vt.astype(BF16_NP)
        lt = np.zeros((128, 384), dtype=BF16_NP)
        rt = np.zeros((128, 768), dtype=BF16_NP)
        for p, (row, c0, c1, _off, g) in enumerate(PIECES[h]):
            slot = LSLOT[p]
            r0, r1 = RCOL[p]
            lt[g:g + K, 128 * slot:128 * (slot + 1)] = \
                a[:, 128 * row:128 * (row + 1)]
            rt[g:g + K, r0:r1] = a[:, c0:c1]
        in_maps.append({"lt": lt, "rt": rt})
    return in_maps


def _assemble(results: list[dict]) -> np.ndarray:
    out = np.empty((B, K, N, N), dtype=np.float32)
    for b in range(B):
        for h in range(2):
            r = results[2 * b + h]["out"].astype(np.float32)   # [K,128,2048]
            re = results[2 * b + h]["oute"].astype(np.float32)  # [8,128,512]
            for p, (row, c0, c1, off, _g) in enumerate(PIECES[h]):
                w = c1 - c0
                dst = out[b, :, 128 * row:128 * (row + 1), c0:c1]
                if off >= 0:
                    dst[:] = r[:, :, off:off + w]
                else:
                    # E piece: pair p2 holds [E(K-1-2p2) | E(K-2-2p2)]
                    for p2 in range(K // 2):
                        out[b, K - 1 - 2 * p2,
                            128 * row:128 * (row + 1), c0:c1] = \
                            re[p2, :, 0:256]
                        out[b, K - 2 - 2 * p2,
                            128 * row:128 * (row + 1), c0:c1] = \
                            re[p2, :, 256:512]
        # mirror the strictly-lower blocks from the computed upper ones
        for i in range(1, 8):
            out[b, :, 128 * i:128 * (i + 1), :128 * i] = np.swapaxes(
                out[b, :, :128 * i, 128 * i:128 * (i + 1)], -1, -2
            )
    return out.reshape(B, K * C, N, N)


def kernel(evecs) -> np.ndarray:
    evecs = np.asarray(evecs, dtype=np.float32)
    assert evecs.shape == (B, C, N, K), evecs.shape
    nc = _get_nc()
    in_maps = _prepare_in_maps(evecs)
    last_err = None
    for _attempt in range(3):
        try:
            r = run_bass_kernel_spmd(nc, in_maps, list(range(NCORES)))
            return _assemble(r.results)
        except Exception as e:  # transient NRT/device hiccups: retry
            last_err = e
    raise last_err


# revision 42
# speedup vs baseline: 1.0042x; 1.0042x over previous
"""Trainium2 Bass kernel for nn_ExpandEvecs.

Computes, for evecs [B=4, C=1, N=1024, K=16]:
    cube[b,l] = V[:, :l+1] @ V[:, :l+1]^T  (Gram expansion per level).

Every level is SYMMETRIC: the device computes only the upper
block-triangle (18 of 64 blocks of 128x128 per core-half) and the host
mirrors the strictly-lower blocks during unsharding. Output stored
bf16, upcast on host.

Architecture (v2): row-tiled PE + copy-balanced evacuation.
Contraction kk = l+1 <= 16 (plain bf16 inputs; the A/B split trick of
v1 is unnecessary: total err 0.19 vs 0.99 allowed), so every matmul
fits one 32-row group of the PE array; tile_position row tiling runs
the four groups' matmuls concurrently, making PE a non-factor (~10us
vs v1's 31us serial column stream). The binding resource is PSUM
evacuation: PSUM has exactly two read ports (Vector @0.96 GHz, Scalar
@1.2 GHz, 1 elem/lane/cycle, no 2x modes reach PSUM on TRN2), so the
2304 f32 cols/level cost ~1.45us/level minimum (scalar: AB 1024-col
copy ~1.11us + amortized E pair ~0.34; vector: C+D ~1.37us). The
orphan 256-col E strip batches ACROSS LEVEL PAIRS into one psum bank
(two matmuls fill halves; one copy + one store per pair into a
separate dram tensor) so its copy cost amortizes. PSUM = 4+2+1+1 = 8
banks exactly (a full level double-buffered needs 9 -- impossible).
Ordering rules (all measured, each worth 1-5us): AB matmuls LEAD the
PE queue (b2 slack means they never wait at the strict-FIFO head;
putting the b1 D chain first costs +5us), C goes last; on the copy
engines the tight b1 copies (E pair on scalar, D on vector) go before
the b2 copies. Stores ride both HWDGE rings (~320 GB/s each),
alternating per level; the last two levels store per-copy to shorten
the drain. Levels are independent fresh matmuls, emitted DESCENDING.

Measured 42.9-43.2us (fast machine state; a bimodal ~+7% slow state
exists) vs the 48.7-48.9us v1 baseline. Anatomy: ~3.3us const-init +
input cold start to first matmul (priority input slices ride the
gpsimd SWDGE queue -- the HWDGE rings dispatch ~1us later behind
framework DRAIN/SET_ORDERING/ACT_TABLE preamble), ~24us copy-bound
level stream at ~1.54us/level (floor 1.45: scalar AB+E2 1.46us,
vector C+D 1.37us), ~4us ring-pipeline store drain, ~8.2us fixed
framework teardown (full semaphore-file sweep, range(150,256),
invariant). E-pair stores must stay on the sync ring: a scalar-ring
dispatch there serializes with the pacing AB copy and costs ~2us
every other pair (measured, fixed).

Tested and rejected: gpsimd SWDGE for bulk stores (slower), fp8
storage (DMA is not the binding resource; PSUM-read cycles are),
E2-copy-after-AB (PE FIFO stall), equal-width 4-chain layouts (a
fully double-buffered level needs 9 banks), wider single-buffered
chains (mm->copy->mm round trip exceeds the 1.45us cadence), C/D
copies ordered after the b2 chains on vector (starves the b1 D
chain), per-block fp8 or diag-block triangle skipping (SPMD shape
mismatch / per-instr overhead exceeds the saving).
"""

import numpy as np
import ml_dtypes

import concourse.mybir as mybir
from concourse import bacc, bass
from concourse.tile import TileContext
from concourse.bass_utils import run_bass_kernel_spmd

B, C, N, K = 4, 1, 1024, 16
NCORES = 8
PACK = 2048            # st columns per level (A|B|C|D)

F32 = mybir.dt.float32
BF16 = mybir.dt.bfloat16
BF16_NP = ml_dtypes.bfloat16

# per core-half: 6 pieces as (block_row, col0, col1, stage_off, row_grp)
# chains: A(0:512)@0, B(512:1024)@32, C(1024:1536)@64,
#         D0(1536:1920)+D1(1920:2048)@96, E(pair-batched)@96
PIECES = [
    [(0, 0, 512, 0, 0), (0, 512, 1024, 512, 32), (4, 512, 1024, 1024, 64),
     (1, 640, 1024, 1536, 96), (3, 896, 1024, 1920, 96),
     (2, 768, 1024, -1, 96)],
    [(1, 128, 640, 0, 0), (2, 256, 768, 512, 32), (3, 384, 896, 1024, 64),
     (5, 640, 1024, 1536, 96), (7, 896, 1024, 1920, 96),
     (6, 768, 1024, -1, 96)],
]
# lhsT column slot per piece (each 128 wide in lt)
LSLOT = [0, 0, 0, 0, 1, 2]
# rhs column range per piece inside rt (per row-group packing)
RCOL = [(0, 512), (0, 512), (0, 512), (0, 384), (384, 512), (512, 768)]

_nc_cache = None


def _build():
    nc = bacc.Bacc(None, target_bir_lowering=False)
    lt_d = nc.declare_dram_parameter("lt", [128, 384], BF16, isOutput=False)
    rt_d = nc.declare_dram_parameter("rt", [128, 768], BF16, isOutput=False)
    out_d = nc.declare_dram_parameter("out", [K, 128, PACK], BF16,
                                      isOutput=True)
    # E columns, one row per level PAIR: [E(level hi) | E(level lo)]
    oute_d = nc.declare_dram_parameter("oute", [K // 2, 128, 512], BF16,
                                       isOutput=True)

    with TileContext(nc) as tc:
        with (
            tc.tile_pool(name="vpool", bufs=1) as vpool,
            tc.tile_pool(name="stage", bufs=6) as stage,
            tc.tile_pool(name="stagee", bufs=4) as stagee,
            tc.tile_pool(name="psum", bufs=1, space=bass.MemorySpace.PSUM) as psum,
        ):
            lt = vpool.tile([128, 384], BF16)
            rt = vpool.tile([128, 768], BF16)
            # split loads: A+B partitions land first so level K-1's
            # first matmuls start earlier. The priority loads ride the
            # gpsimd SWDGE queue; the HWDGE rings carry the rest and
            # arrive in time for the later chains.
            nc.gpsimd.dma_start(out=lt[0:48, :], in_=lt_d[0:48, :])
            nc.gpsimd.dma_start(out=rt[0:48, :], in_=rt_d[0:48, :])
            nc.sync.dma_start(out=rt[64:112, :], in_=rt_d[64:112, :])
            nc.scalar.dma_start(out=lt[64:112, :], in_=lt_d[64:112, :])

            psE = None
            stE = None
            for i, l in enumerate(range(K - 1, -1, -1)):
                kk = l + 1
                st = stage.tile([128, PACK], BF16, tag="st", name=f"st{l}")
                psAB = psum.tile([128, 1024], F32, tag="psAB", bufs=2,
                                 name=f"psAB{l}")
                psC = psum.tile([128, 512], F32, tag="psC", bufs=2,
                                name=f"psC{l}")
                psD = psum.tile([128, 512], F32, tag="psD", bufs=1,
                                name=f"psD{l}")
                if i % 2 == 0:
                    psE = psum.tile([128, 512], F32, tag="psE", bufs=1,
                                    name=f"psE{l}")
                    stE = stagee.tile([128, 512], BF16, tag="stE",
                                      name=f"stE{l}")

                def mm(ps, slot, r0, r1, g):
                    nc.tensor.matmul(
                        ps,
                        lhsT=lt[g:g + kk, 128 * slot:128 * (slot + 1)],
                        rhs=rt[g:g + kk, r0:r1],
                        start=True, stop=True,
                        tile_position=(g, 0),
                    )

                # PE issue order: AB leads (it feeds the scalar pacer
                # and its b2 slack means it never waits at the FIFO
                # head), then the tight b1 chains D/E, C last. Putting
                # D first measures +5us: its b1 wait at the FIFO head
                # blocks the AB matmuls behind it every level.
                mm(psAB[:, 0:512], 0, 0, 512, 0)
                mm(psAB[:, 512:1024], 0, 0, 512, 32)
                mm(psD[:, 0:384], 0, 0, 384, 96)
                mm(psD[:, 384:512], 1, 384, 512, 96)
                eoff = 256 * (i % 2)
                mm(psE[:, eoff:eoff + 256], 2, 512, 768, 96)
                mm(psC[:, :], 0, 0, 512, 64)

                # PSUM->SBUF bf16 casts: scalar AB then E pair, vector
                # D then C. AB first on scalar: the E-pair copy is
                # gated by the odd level's E matmul (5th in PE order),
                # and putting it first makes the pacer inherit that
                # wait (~150ns/pair bubble). The next even level's E
                # matmul still clears: E2 ends ~2.0us into the pair vs
                # a ~2.4us deadline.
                nc.scalar.copy(st[:, 0:1024], psAB[:])
                if i % 2 == 1:
                    nc.scalar.copy(stE[:], psE[:])
                nc.vector.tensor_copy(st[:, 1536:2048], psD[:])
                nc.vector.tensor_copy(st[:, 1024:1536], psC[:])

                # stores alternate rings per level for byte balance;
                # first and last levels store per-copy (fast ramp-in,
                # short drain)
                r_ab = nc.sync if i % 2 == 0 else nc.scalar
                r_cd = nc.scalar if i % 2 == 0 else nc.sync
                if i == K - 1:
                    # final level: 512-col chunks alternating rings so
                    # neither ring pipelines a long last transfer
                    for s in range(4):
                        ring = nc.sync if s % 2 == 0 else nc.scalar
                        ring.dma_start(out=out_d[l, :, 512 * s:512 * (s + 1)],
                                       in_=st[:, 512 * s:512 * (s + 1)])
                elif i == K - 2:
                    r_ab.dma_start(out=out_d[l, :, 0:1024],
                                   in_=st[:, 0:1024])
                    r_cd.dma_start(out=out_d[l, :, 1024:1536],
                                   in_=st[:, 1024:1536])
                    r_ab.dma_start(out=out_d[l, :, 1536:2048],
                                   in_=st[:, 1536:2048])
                else:
                    r_ab.dma_start(out=out_d[l, :, 0:1024],
                                   in_=st[:, 0:1024])
                    r_cd.dma_start(out=out_d[l, :, 1024:2048],
                                   in_=st[:, 1024:2048])

                if i % 2 == 1:
                    # close the E pair: one store (copy emitted above).
                    # Always the sync ring: a scalar-ring dispatch here
                    # serializes with the pacing AB copy (measured ~2us
                    # hiccup). Final pair splits across rings (past the
                    # last AB copy, no hazard).
                    if i == K - 1:
                        nc.sync.dma_start(out=oute_d[i // 2, :, 0:256],
                                          in_=stE[:, 0:256])
                        nc.scalar.dma_start(out=oute_d[i // 2, :, 256:512],
                                            in_=stE[:, 256:512])
                    else:
                        nc.sync.dma_start(out=oute_d[i // 2], in_=stE[:])

    nc.compile()
    return nc


def _get_nc():
    global _nc_cache
    if _nc_cache is None:
        _nc_cache = _build()
    return _nc_cache


def _prepare_in_maps(evecs: np.ndarray) -> list[dict]:
    in_maps = []
    for c in range(NCORES):
        b, h = divmod(c, 2)
        vt = np.ascontiguousarray(evecs[b, 0].T, dtype=np.float32)  # [K, N]
        a = vt.astype(BF16_NP)
        lt = np.zeros((128, 384), dtype=BF16_NP)
        rt = np.zeros((128, 768), dtype=BF16_NP)
        for p, (row, c0, c1, _off, g) in enumerate(PIECES[h]):
            slot = LSLOT[p]
            r0, r1 = RCOL[p]
            lt[g:g + K, 128 * slot:128 * (slot + 1)] = \
                a[:, 128 * row:128 * (row + 1)]
            rt[g:g + K, r0:r1] = a[:, c0:c1]
        in_maps.append({"lt": lt, "rt": rt})
    return in_maps


def _assemble(results: list[dict]) -> np.ndarray:
    out = np.empty((B, K, N, N), dtype=np.float32)
    for b in range(B):
        for h in range(2):
            r = results[2 * b + h]["out"].astype(np.float32)   # [K,128,2048]
            re = results[2 * b + h]["oute"].astype(np.float32)  # [8,128,512]
            for p, (row, c0, c1, off, _g) in enumerate(PIECES[h]):
                w = c1 - c0
                dst = out[b, :, 128 * row:128 * (row + 1), c0:c1]
                if off >= 0:
                    dst[:] = r[:, :, off:off + w]
                else:
                    # E piece: pair p2 holds [E(K-1-2p2) | E(K-2-2p2)]
                    for p2 in range(K // 2):
                        out[b, K - 1 - 2 * p2,
                            128 * row:128 * (row + 1), c0:c1] = \
                            re[p2, :, 0:256]
                        out[b, K - 2 - 2 * p2,
                            128 * row:128 * (row + 1), c0:c1] = \
                            re[p2, :, 256:512]
        # mirror the strictly-lower blocks from the computed upper ones
        for i in range(1, 8):
            out[b, :, 128 * i:128 * (i + 1), :128 * i] = np.swapaxes(
                out[b, :, :128 * i, 128 * i:128 * (i + 1)], -1, -2
            )
    return out.reshape(B, K * C, N, N)


def kernel(evecs) -> np.ndarray:
    evecs = np.asarray(evecs, dtype=np.float32)
    assert evecs.shape == (B, C, N, K), evecs.shape
    nc = _get_nc()
    in_maps = _prepare_in_maps(evecs)
    last_err = None
    for _attempt in range(3):
        try:
            r = run_bass_kernel_spmd(nc, in_maps, list(range(NCORES)))
            return _assemble(r.results)
        except Exception as e:  # transient NRT/device hiccups: retry
            last_err = e
    raise last_err
